# revision 39
# baseline (speedup 1.0000x reference)
"""Trainium2 Bass kernel for Bahdanau-style attention (nn_AttentionLayer).

reference:
    proj_f = features @ W1_w + W1_b          # [B,T,U]
    proj_h = (hidden @ W2_w + W2_b)[:,None]  # [B,1,U]
    score  = tanh(proj_f + proj_h)           # [B,T,U]
    logits = score @ V_w + V_b               # [B,T,1]
    attn   = softmax(logits, axis=1)         # [B,T,1]
    ctx    = sum(attn * features, axis=1)    # [B,D]
    return ctx, attn

B=64, T=1024, D=512, U=1024. Data-parallel over batch: 8 examples per core
on 8 NeuronCores. V_b cancels inside the softmax, so it is ignored.

Matmuls run in float32r (TF32-like) mode: full 1 cyc/row rate at N=512 vs
4 cyc/row for plain fp32. The walrus verifier requires every f32r matmul
operand to be produced rounded-to-f32r, so DMA-loaded tiles get a one-time
DVE cast-copy; compute-produced tiles (scores, fT, wT) are written as f32r
directly by their producing instruction.

Per-core dataflow (example b, half h of T):
  - features[b] loaded once in natural [t,d] tiles, PE-transposed on chip
    to [d,t] tiles for the main matmul (fp32 DMA-transpose doesn't exist).
  - main matmul: proj[u:128, t:512] += W1[d:128, u:128].T @ fT[d:128, t:512]
  - ScalarE tanh with per-partition bias = proj_h[b] + W1_b + W2_b (fused).
  - logits[1, t:512] += V[u:128, 1].T @ score[u:128, t:512]      (8 u-chunks)
  - softmax over T on the [1, 1024] row: Exp with accum_out (denominator),
    reciprocal, scale. No max subtraction: |logit| <= sum|V| ~ 26, exp fits
    fp32 with headroom and matches the reference exactly in exact math.
  - weights transposed to [t:128, 8] via 8 K=1 matmuls, then
    ctx[1, d:512] += wT[t:128, 1].T @ f_nat[t:128, d:512]        (8 t-chunks)
"""

import numpy as np

import concourse.bass as bass
import concourse.tile as tile
from bass_rust import add_dep_helper
from concourse import masks, mybir
from concourse.bass_utils import run_bass_kernel_spmd

B, T, D, U = 64, 1024, 512, 512 * 2  # U = 1024
NCORES = 8
BC = B // NCORES  # 8 examples per core
TH = 512          # T half
NDC = D // 128    # 4 d-chunks
NUC = U // 128    # 8 u-chunks
F32 = mybir.dt.float32
F32R = mybir.dt.float32r

_CACHE = {}


def build_graph(use_f32r=True):
    nc = bass.Bass()
    DTM = F32R if use_f32r else F32

    feats = nc.declare_dram_parameter("features", [BC, T, D], F32, isOutput=False)
    hidden = nc.declare_dram_parameter("hidden", [BC, D], F32, isOutput=False)
    w1 = nc.declare_dram_parameter("W1_w", [D, U], F32, isOutput=False)
    w1b = nc.declare_dram_parameter("W1_b", [U], F32, isOutput=False)
    w2 = nc.declare_dram_parameter("W2_w", [D, U], F32, isOutput=False)
    w2b = nc.declare_dram_parameter("W2_b", [U], F32, isOutput=False)
    vw = nc.declare_dram_parameter("V_w", [U, 1], F32, isOutput=False)
    out_ctx = nc.declare_dram_parameter("out_ctx", [BC, D], F32, isOutput=True)
    ctx_scr = nc.dram_tensor("ctx_scr", [BC, 256], F32)
    out_attn = nc.declare_dram_parameter("out_attn", [BC, T], F32, isOutput=True)

    with tile.TileContext(nc) as tc:
        with (
            tc.tile_pool(name="const", bufs=1) as constp,
            tc.tile_pool(name="fnat", bufs=4) as fnatp,
            tc.tile_pool(name="fnr", bufs=4) as fnrp,
            tc.tile_pool(name="ft", bufs=2) as ftp,
            tc.tile_pool(name="score", bufs=2) as scorep,
            tc.tile_pool(name="rows", bufs=1) as rowsp,
            tc.tile_pool(name="ptr", bufs=1, space=bass.MemorySpace.PSUM) as ptrp,
            tc.tile_pool(name="pp", bufs=2, space=bass.MemorySpace.PSUM) as ppp,
            tc.tile_pool(name="pl", bufs=2, space=bass.MemorySpace.PSUM) as plp,
            tc.tile_pool(name="psm", bufs=2, space=bass.MemorySpace.PSUM) as psmp,
        ):
            # ---- long-lived constants ----
            identc = constp.tile([128, 128], DTM)
            zerocol = constp.tile([128, 1], DTM)
            zerocol_f = constp.tile([128, 1], F32)
            nc.vector.memset(zerocol_f[:], 0.0)
            nc.vector.tensor_copy(zerocol[:], zerocol_f[:])
            dve_dummy = constp.tile([1, 16], F32)
            zrow = constp.tile([1, 128], F32)
            nc.vector.memset(zrow[:], 0.0)
            ones_ad = constp.tile([1, 2], F32)
            nc.vector.memset(ones_ad[:], 1.0)
            ones1 = ones_ad[:, 0:1]
            w1_r = constp.tile([128, NDC, U], DTM)
            v_r = constp.tile([128, NUC, 1], DTM)
            bias_sb = constp.tile([128, NUC, BC], F32)
            phc = constp.tile([128, NUC, BC], F32)

            # ---- setup (pool released before the main loop frees its SBUF) ----
            with tc.tile_pool(name="setup", bufs=1) as setupp:
                ident = setupp.tile([128, 128], F32)
                masks.make_identity(nc, ident[:])
                nc.vector.tensor_copy(identc[:], ident[:])

                # W1 as [128(d_in_chunk), dc, u]; rounded copy for matmul use
                w1_sb = setupp.tile([128, NDC, U], F32)
                nc.sync.dma_start(
                    w1_sb[:], w1.ap().rearrange("(dc p) u -> p dc u", p=128)
                )
                nc.vector.tensor_copy(w1_r[:], w1_sb[:])

                w2_sb = setupp.tile([128, NDC, U], F32)
                nc.sync.dma_start(
                    w2_sb[:], w2.ap().rearrange("(dc p) u -> p dc u", p=128)
                )
                # V as [128(u_in_chunk), uc, 1]; rounded copy
                v_sb = setupp.tile([128, NUC, 1], F32)
                nc.sync.dma_start(
                    v_sb[:], vw.ap().rearrange("(c p) o -> p c o", p=128)
                )
                nc.vector.tensor_copy(v_r[:], v_sb[:])

                # W1_b + W2_b as [128(u_in_chunk), uc]
                b1t = setupp.tile([128, NUC], F32)
                nc.sync.dma_start(b1t[:], w1b.ap().rearrange("(c p) -> p c", p=128))
                b2t = setupp.tile([128, NUC], F32)
                nc.sync.dma_start(b2t[:], w2b.ap().rearrange("(c p) -> p c", p=128))
                b12a = setupp.tile([128, NUC], F32)
                nc.vector.tensor_copy(b12a[:], b1t[:])
                b12b = setupp.tile([128, NUC], F32)
                nc.vector.tensor_copy(b12b[:], b2t[:])
                b12 = setupp.tile([128, NUC], F32)
                nc.vector.tensor_add(b12[:], b12a[:], b12b[:])

                # hidden^T as [128(d_in_chunk), dc, b]
                h_t = setupp.tile([128, NDC, BC], F32)
                for dc in range(NDC):
                    nc.sync.dma_start(
                        h_t[:, dc, :],
                        hidden.ap()[:, dc * 128 : (dc + 1) * 128].rearrange(
                            "b p -> p b"
                        ),
                    )

                w2c = setupp.tile([128, NDC, U], F32)
                nc.vector.tensor_copy(w2c[:], w2_sb[:])
                h_tc = setupp.tile([128, NDC, BC], F32)
                for dc in range(NDC):
                    nc.vector.tensor_copy(h_tc[:, dc, :], h_t[:, dc, :])

                # bias[u, b] = proj_h[b, u] + W1_b[u] + W2_b[u]  (fp32 matmul)
                for u in range(NUC):
                    ph = psmp.tile([128, BC], F32, tag="small")
                    for dc in range(NDC):
                        nc.tensor.matmul(
                            ph[:],
                            w2c[:, dc, u * 128 : (u + 1) * 128],
                            h_tc[:, dc, :],
                            start=(dc == 0),
                            stop=(dc == NDC - 1),
                        )
                    nc.vector.tensor_copy(phc[:, u, :], ph[:])
                    nc.vector.tensor_add(
                        bias_sb[:, u, :],
                        phc[:, u, :],
                        b12[:, u : u + 1].to_broadcast((128, BC)),
                    )

            # ACT absorber: make ScalarE observe the DVE tick of bias_sb so
            # every tanh carries only the PE wait (ISA structs hold one wait).
            bias_ab = nc.scalar.activation(
                ones_ad[:, 1:2], bias_sb[0:1, NUC - 1, 0:1],
                mybir.ActivationFunctionType.Identity,
            )

            # ---- main loop ----
            # Software-pipelined: half k+1's features load + rounding copy are
            # issued before half k's compute, so (a) the DVE tick of fr(k) is
            # already PE-observed when the transposes of half k issue (keeps
            # them at one wait), and (b) DMA overlaps compute.
            prev_attn_dma = None
            prev_zmm = [None]
            prev_sc = [None]
            frs = {}
            ft_last = {}       # half -> last ft copy inst
            ctx_dmas = {}      # example -> out_ctx dma inst
            _ab = [0]

            def dve_absorber(target_inst, reason):
                # A 1-element DVE copy that carries a wait the next copy's
                # ISA slot cannot hold. Rotating write slots keep its own
                # WAW ancient (and thus elided).
                i = _ab[0] % 16
                _ab[0] += 1
                inst = nc.vector.tensor_copy(
                    dve_dummy[0:1, i : i + 1], ones_ad[0:1, 0:1]
                )
                add_dep_helper(inst.ins, target_inst.ins, sync=True, reason=reason)
                return inst

            def load_half(k):
                bb, hh = divmod(k, 2)
                fn = fnatp.tile([128, 4, D], F32)
                fn_dma = nc.gpsimd.dma_start(
                    fn[:],
                    feats.ap()[bb, hh * TH : (hh + 1) * TH, :].rearrange(
                        "(c p) d -> p c d", p=128
                    ),
                )
                if prev_attn_dma is not None:
                    # SP-order the prefetch after the previous example's attn
                    # store: its DVE wait covers this slot's WAR, so this DMA
                    # carries only DMAHW waits (struct holds 2).
                    add_dep_helper(
                        fn_dma.ins, prev_attn_dma.ins, sync=False,
                        reason="order fn prefetch after prev attn out",
                    )
                fr = fnrp.tile([128, 4, D], DTM)
                nc.vector.tensor_copy(fr[:], fn[:])
                frs[k] = fr

            load_half(0)
            load_half(1)
            for b in range(BC):
                logit_row = rowsp.tile([1, T], F32, tag="logit")
                for h in range(2):
                    k = b * 2 + h
                    if k + 2 < BC * 2:
                        load_half(k + 2)
                    fr = frs[k]

                    def tr_group(dc):
                        # 4 transposes [t:128, d:128] -> ptr[:, 128] quarters
                        ptr = ptrp.tile([128, TH], DTM, tag=f"ptr{dc % 2}")
                        first = None
                        for c in range(4):
                            inst = nc.tensor.transpose(
                                ptr[:, c * 128 : (c + 1) * 128],
                                fr[:, c, dc * 128 : (dc + 1) * 128],
                                identc[:],
                            )
                            if first is None:
                                first = inst
                        ft = ftp.tile([128, TH], DTM, tag=f"ft{dc}")
                        ftc = nc.vector.tensor_copy(ft[:], ptr[:])
                        if ftab is not None:
                            add_dep_helper(
                                ftc.ins, ftab.ins, sync=False,
                                reason="DVE-order ft copy after absorber",
                            )
                        if dc == 3:
                            ft_last[k] = ftc
                        return ft, first

                    def mm_u0(dc, pp, fts):
                        return nc.tensor.matmul(
                            pp[:],
                            w1_r[:, dc, u0 * 128 : (u0 + 1) * 128],
                            fts[dc][:],
                            start=(dc == 0),
                            stop=False,
                        )

                    # Interleave: transposes of dc2/dc3 are PE-ordered after
                    # the first main matmuls, whose ft waits make their fr/ft
                    # WAR ticks already-observed (ISA structs hold one wait).
                    ftab = None
                    if k - 2 in ft_last:
                        ftab = dve_absorber(
                            ft_last[k - 2], "DVE observes ft copies of k-2"
                        )
                    fts = []
                    u0 = 0
                    for dc in (0, 1):
                        ft, tr_first = tr_group(dc)
                        fts.append(ft)
                        if prev_zmm[0] is not None:
                            add_dep_helper(
                                tr_first.ins, prev_zmm[0].ins, sync=False,
                                reason="PE-order transposes after prev zero-mm",
                            )
                    pp0 = ppp.tile([128, TH], F32, tag="pp")
                    mm00 = mm_u0(0, pp0, fts)
                    mm01 = mm_u0(1, pp0, fts)
                    for dc, mm in ((2, mm00), (3, mm01)):
                        ft, tr_first = tr_group(dc)
                        fts.append(ft)
                        add_dep_helper(
                            tr_first.ins, mm.ins, sync=False,
                            reason="PE-order transposes after u0 matmul",
                        )
                    # proj + tanh + V-reduce. The logit accumulation opens
                    # with a zero matmul (0.T @ fr of half k+1): it clears the
                    # PSUM bank, absorbs the slot WAR, and makes PE observe
                    # fr(k+1)'s DVE tick so the next half's transposes fit in
                    # their single wait slot.
                    pl = plp.tile([1, TH], F32)
                    vstart = True
                    if k + 1 < BC * 2:
                        zmm = nc.tensor.matmul(
                            pl[:],
                            zerocol[:],
                            frs[k + 1][:, 0, :],
                            start=True,
                            stop=False,
                        )
                        prev_zmm[0] = zmm
                        vstart = False
                    for u in range(NUC):
                        pp = pp0 if u == 0 else ppp.tile([128, TH], F32, tag="pp")
                        for dc in range(NDC):
                            if u == 0 and dc < 2:
                                continue
                            nc.tensor.matmul(
                                pp[:],
                                w1_r[:, dc, u * 128 : (u + 1) * 128],
                                fts[dc][:],
                                start=(u != 0 and dc == 0),
                                stop=(dc == NDC - 1),
                            )
                        sc = scorep.tile([128, TH], DTM)
                        if u % 2 == 0 and prev_sc[0] is not None:
                            # ACT absorber: carries the score-slot self-WAW
                            # (via an explicit sync dep on the previous tanh,
                            # reading no score tile) so each tanh keeps only
                            # its PE wait.
                            ab = nc.scalar.activation(
                                ones_ad[0:1, 1:2],
                                ones_ad[0:1, 0:1],
                                mybir.ActivationFunctionType.Identity,
                            )
                            add_dep_helper(
                                ab.ins, prev_sc[0].ins, sync=True,
                                reason="ACT observes prev tanh tick",
                            )
                        tanh_inst = nc.scalar.activation(
                            sc[:],
                            pp[:],
                            mybir.ActivationFunctionType.Tanh,
                            bias=bias_sb[:, u, b : b + 1],
                        )
                        if prev_sc[0] is None:
                            add_dep_helper(
                                tanh_inst.ins, bias_ab.ins, sync=False,
                                reason="ACT-order first tanh after bias absorber",
                            )
                        prev_sc[0] = tanh_inst
                        nc.tensor.matmul(
                            pl[:],
                            v_r[:, u, :],
                            sc[:],
                            start=(u == 0 and vstart),
                            stop=(u == NUC - 1),
                        )
                    nc.vector.tensor_copy(logit_row[:, h * TH : (h + 1) * TH], pl[:])

                # softmax over T (no max-subtract needed; |logit| <= sum|V|)
                exp_row = rowsp.tile([1, T], F32, tag=f"exp{b % 2}")
                sr = rowsp.tile([1, 2], F32, tag=f"sr{b % 2}")
                esum = sr[:, 0:1]
                rinv = sr[:, 1:2]
                nc.scalar.activation(
                    exp_row[:],
                    logit_row[:],
                    mybir.ActivationFunctionType.Exp,
                    accum_out=esum,
                )
                exp_c = rowsp.tile([1, T], F32, tag=f"expc{b % 2}")
                nc.vector.tensor_copy(exp_c[:], exp_row[:])
                nc.vector.reciprocal(rinv, esum)
                w_row = rowsp.tile([1, T], F32, tag=f"wrow{b}")
                nc.vector.tensor_mul(w_row[:], exp_c[:], rinv.to_broadcast((1, T)))
                # interleaved 3D AP: the 3D pseudo-DMA struct holds two
                # waits (producer + lane); the contiguous 2D one holds one.
                prev_attn_dma = nc.gpsimd.dma_start(
                    out_attn.ap()[b : b + 1, :], w_row[:]
                )

                # transpose weights to [t:128, chunk:8] via K=1 matmuls.
                # A zero matmul (0.T @ zeros) opens the accumulation group:
                # it clears the bank and carries the slot WAW, so the first
                # data matmul waits only on w_row's DVE tick.
                pw = psmp.tile([128, 8], F32, tag="small")
                zpw = nc.tensor.matmul(
                    pw[:],
                    zrow[0:1, 0:128],
                    zrow[0:1, 0:8],
                    start=True,
                    stop=False,
                )
                first_wmm = None
                for c in range(8):
                    wmm = nc.tensor.matmul(
                        pw[:, c : c + 1],
                        w_row[0:1, c * 128 : (c + 1) * 128],
                        ones1,
                        start=False,
                        stop=(c == 7),
                    )
                    if first_wmm is None:
                        first_wmm = wmm
                        add_dep_helper(
                            wmm.ins, zpw.ins, sync=False,
                            reason="PE-order w-transpose after zero open",
                        )
                wt = rowsp.tile([128, 8], DTM, tag=f"wt{b % 2}")
                nc.vector.tensor_copy(wt[:], pw[:])

                # context: ctx[1, 512] += wT[t,1].T @ f_nat[t:128, d:512],
                # opened by a zero matmul (0.T @ logit_row slice) whose DVE
                # wait merges with the data matmuls' wt wait.
                pc = psmp.tile([1, D], F32, tag="small")
                zpc = nc.tensor.matmul(
                    pc[:],
                    zrow[0:1, 0:1],
                    logit_row[0:1, 0:D],
                    start=True,
                    stop=False,
                )
                first_cmm = None
                for c in range(8):
                    cmm = nc.tensor.matmul(
                        pc[:],
                        wt[:, c : c + 1],
                        frs[b * 2 + c // 4][:, c % 4, :],
                        start=False,
                        stop=(c == 7),
                    )
                    if first_cmm is None:
                        first_cmm = cmm
                        add_dep_helper(
                            cmm.ins, zpc.ins, sync=False,
                            reason="PE-order ctx after zero open",
                        )
                ctx_row = rowsp.tile([1, D], F32, tag=f"ctxrow{b % 2}")
                if b - 2 in ctx_dmas:
                    cab = dve_absorber(
                        ctx_dmas[b - 2], "DVE observes out_ctx dma of b-2"
                    )
                    crc = nc.vector.tensor_copy(ctx_row[:], pc[:])
                    add_dep_helper(
                        crc.ins, cab.ins, sync=False,
                        reason="DVE-order ctx_row copy after absorber",
                    )
                else:
                    nc.vector.tensor_copy(ctx_row[:], pc[:])
                carrier = nc.gpsimd.dma_start(
                    ctx_scr.ap()[b : b + 1, :].rearrange("o (c p) -> o c p", p=64),
                    ctx_row.rearrange("o (c q) -> o c q", q=128)[:, :, 0:64],
                )
                ctx_dmas[b] = nc.gpsimd.dma_start(
                    out_ctx.ap()[b : b + 1, :], ctx_row[:]
                )
                add_dep_helper(
                    ctx_dmas[b].ins, carrier.ins, sync=False,
                    reason="Pool-order ctx store after carrier (ctx_row observed)",
                )
                frs.pop(b * 2, None)
                frs.pop(b * 2 + 1, None)

    return nc


def _split_excess_waits(nc, cap=1):
    """Walrus ISA structs hold very few semaphore waits per instruction
    (1 for most opcodes). Hoist all but `cap` waits of any instruction onto
    same-engine NoOps inserted immediately before it."""
    ctr = 0
    for fn in nc.m.functions:
        for blk in fn.blocks:
            insts = blk.instructions
            i = 0
            while i < len(insts):
                inst = insts[i]
                si = inst.sync_info
                if si is not None and si.on_wait and len(si.on_wait) > cap:
                    waits = list(si.on_wait)
                    for w in waits[:-cap]:
                        ctr += 1
                        nop = mybir.InstNoOp(
                            name=f"waitnop-{ctr}",
                            ins=[],
                            outs=[],
                            engine=inst.engine,
                            sync_info=mybir.SyncInfo(on_wait=[w], on_update=[]),
                        )
                        insts.insert(i, nop)
                        i += 1
                    inst.sync_info = mybir.SyncInfo(
                        on_wait=waits[-cap:], on_update=list(si.on_update)
                    )
                i += 1
    return ctr


def _get_graph(**kw):
    key = tuple(sorted(kw.items()))
    if key not in _CACHE:
        nc = build_graph(**kw)
        n = _split_excess_waits(nc)
        _CACHE[key] = nc
    return _CACHE[key]


def kernel(features, hidden, W1_w, W1_b, W2_w, W2_b, V_w, V_b=None, **ignored):
    nc = _get_graph()
    shared = {
        "W1_w": np.ascontiguousarray(W1_w, dtype=np.float32),
        "W1_b": np.ascontiguousarray(W1_b, dtype=np.float32),
        "W2_w": np.ascontiguousarray(W2_w, dtype=np.float32),
        "W2_b": np.ascontiguousarray(W2_b, dtype=np.float32),
        "V_w": np.ascontiguousarray(V_w, dtype=np.float32),
    }
    in_maps = []
    for c in range(NCORES):
        m = dict(shared)
        m["features"] = np.ascontiguousarray(
            features[c * BC : (c + 1) * BC], dtype=np.float32
        )
        m["hidden"] = np.ascontiguousarray(
            hidden[c * BC : (c + 1) * BC], dtype=np.float32
        )
        in_maps.append(m)
    res = run_bass_kernel_spmd(nc, in_maps, core_ids=list(range(NCORES)))
    ctx = np.concatenate([res.results[i]["out_ctx"] for i in range(NCORES)], axis=0)
    attn = np.concatenate(
        [res.results[i]["out_attn"] for i in range(NCORES)], axis=0
    ).reshape(B, T, 1)
    return ctx.astype(np.float32), attn.astype(np.float32)


# revision 44
# speedup vs baseline: 1.0996x; 1.0996x over previous
"""Trainium2 Bass kernel for Bahdanau-style attention (nn_AttentionLayer).

reference:
    proj_f = features @ W1_w + W1_b          # [B,T,U]
    proj_h = (hidden @ W2_w + W2_b)[:,None]  # [B,1,U]
    score  = tanh(proj_f + proj_h)           # [B,T,U]
    logits = score @ V_w + V_b               # [B,T,1]
    attn   = softmax(logits, axis=1)         # [B,T,1]
    ctx    = sum(attn * features, axis=1)    # [B,D]
    return ctx, attn

B=64, T=1024, D=512, U=1024. Data-parallel over batch: 8 examples per core
on 8 NeuronCores. V_b cancels inside the softmax, so it is ignored.

Matmuls run in float32r (TF32-like) mode: full 1 cyc/row rate at N=512 vs
4 cyc/row for plain fp32. The walrus verifier requires every f32r matmul
operand to be produced rounded-to-f32r, so DMA-loaded tiles get a one-time
DVE cast-copy; compute-produced tiles (scores, fT, wT) are written as f32r
directly by their producing instruction.

Per-core dataflow (example b, half h of T):
  - features[b] loaded once in natural [t,d] tiles, PE-transposed on chip
    to [d,t] tiles for the main matmul (fp32 DMA-transpose doesn't exist).
  - main matmul: proj[u:128, t:512] += W1[d:128, u:128].T @ fT[d:128, t:512]
  - ScalarE tanh with per-partition bias = proj_h[b] + W1_b + W2_b (fused).
  - logits[1, t:512] += V[u:128, 1].T @ score[u:128, t:512]      (8 u-chunks)
  - softmax over T on the [1, 1024] row: Exp with accum_out (denominator),
    reciprocal, scale. No max subtraction: |logit| <= sum|V| ~ 26, exp fits
    fp32 with headroom and matches the reference exactly in exact math.
  - weights transposed to [t:128, 8] via 8 K=1 matmuls, then
    ctx[1, d:512] += wT[t:128, 1].T @ f_nat[t:128, d:512]        (8 t-chunks)
"""

import numpy as np

import concourse.bass as bass
import concourse.tile as tile
from bass_rust import add_dep_helper
from concourse import masks, mybir
from concourse.bass_utils import run_bass_kernel_spmd

B, T, D, U = 64, 1024, 512, 512 * 2  # U = 1024
NCORES = 8
BC = B // NCORES  # 8 examples per core
TH = 512          # T half
NDC = D // 128    # 4 d-chunks
NUC = U // 128    # 8 u-chunks
F32 = mybir.dt.float32
F32R = mybir.dt.float32r
BF16 = mybir.dt.bfloat16

_CACHE = {}


def build_graph(use_f32r=True):
    nc = bass.Bass()
    DTM = F32R if use_f32r else F32

    feats = nc.declare_dram_parameter("features", [BC, T, D], F32, isOutput=False)
    hidden = nc.declare_dram_parameter("hidden", [BC, D], F32, isOutput=False)
    w1 = nc.declare_dram_parameter("W1_w", [D, U], F32, isOutput=False)
    w1b = nc.declare_dram_parameter("W1_b", [U], F32, isOutput=False)
    w2 = nc.declare_dram_parameter("W2_w", [D, U], F32, isOutput=False)
    w2b = nc.declare_dram_parameter("W2_b", [U], F32, isOutput=False)
    vw = nc.declare_dram_parameter("V_w", [U, 1], F32, isOutput=False)
    out_ctx = nc.declare_dram_parameter("out_ctx", [BC, D], F32, isOutput=True)
    ctx_scr = nc.dram_tensor("ctx_scr", [BC, 256], F32)
    out_attn = nc.declare_dram_parameter("out_attn", [BC, T], F32, isOutput=True)

    with tile.TileContext(nc) as tc:
        with (
            tc.tile_pool(name="const", bufs=1) as constp,
            tc.tile_pool(name="fnat", bufs=4) as fnatp,
            tc.tile_pool(name="fnr", bufs=4) as fnrp,
            tc.tile_pool(name="ft", bufs=2) as ftp,
            tc.tile_pool(name="score", bufs=2) as scorep,
            tc.tile_pool(name="rows", bufs=1) as rowsp,
            tc.tile_pool(name="ptr", bufs=1, space=bass.MemorySpace.PSUM) as ptrp,
            tc.tile_pool(name="pp", bufs=2, space=bass.MemorySpace.PSUM) as ppp,
            tc.tile_pool(name="pl", bufs=2, space=bass.MemorySpace.PSUM) as plp,
            tc.tile_pool(name="psm", bufs=2, space=bass.MemorySpace.PSUM) as psmp,
        ):
            # ---- long-lived constants ----
            identc = constp.tile([128, 128], DTM)
            zerocol = constp.tile([128, 1], DTM)
            zerocol_f = constp.tile([128, 1], F32)
            nc.vector.memset(zerocol_f[:], 0.0)
            nc.vector.tensor_copy(zerocol[:], zerocol_f[:])
            dve_dummy = constp.tile([1, 16], F32)
            zrow_f = constp.tile([1, D], F32)
            nc.vector.memset(zrow_f[:], 0.0)
            zrow = constp.tile([1, D], DTM)
            nc.vector.tensor_copy(zrow[:], zrow_f[:])
            ones1_b = constp.tile([1, 1], BF16)
            zrow_b = constp.tile([1, D], BF16)
            nc.vector.tensor_copy(zrow_b[:], zrow_f[:])
            ones_ad = constp.tile([1, 2], F32)
            nc.vector.memset(ones_ad[:], 1.0)
            ones1 = ones_ad[:, 0:1]
            nc.vector.tensor_copy(ones1_b[:], ones1)
            w1_r = constp.tile([128, NDC, U], DTM)
            v_r = constp.tile([128, NUC, 1], DTM)
            bias_sb = constp.tile([128, NUC, BC], F32)
            phc = constp.tile([128, NUC, BC], F32)

            # ---- setup (pool released before the main loop frees its SBUF) ----
            with tc.tile_pool(name="setup", bufs=1) as setupp:
                ident = setupp.tile([128, 128], F32)
                masks.make_identity(nc, ident[:])
                nc.vector.tensor_copy(identc[:], ident[:])

                # W1 as [128(d_in_chunk), dc, u]; rounded copy for matmul use
                w1_sb = setupp.tile([128, NDC, U], F32)
                nc.sync.dma_start(
                    w1_sb[:], w1.ap().rearrange("(dc p) u -> p dc u", p=128)
                )
                nc.vector.tensor_copy(w1_r[:], w1_sb[:])

                w2_sb = setupp.tile([128, NDC, U], F32)
                nc.sync.dma_start(
                    w2_sb[:], w2.ap().rearrange("(dc p) u -> p dc u", p=128)
                )
                # V as [128(u_in_chunk), uc, 1]; rounded copy
                v_sb = setupp.tile([128, NUC, 1], F32)
                nc.sync.dma_start(
                    v_sb[:], vw.ap().rearrange("(c p) o -> p c o", p=128)
                )
                nc.vector.tensor_copy(v_r[:], v_sb[:])

                # W1_b + W2_b as [128(u_in_chunk), uc]
                b1t = setupp.tile([128, NUC], F32)
                nc.sync.dma_start(b1t[:], w1b.ap().rearrange("(c p) -> p c", p=128))
                b2t = setupp.tile([128, NUC], F32)
                nc.sync.dma_start(b2t[:], w2b.ap().rearrange("(c p) -> p c", p=128))
                b12a = setupp.tile([128, NUC], F32)
                nc.vector.tensor_copy(b12a[:], b1t[:])
                b12b = setupp.tile([128, NUC], F32)
                nc.vector.tensor_copy(b12b[:], b2t[:])
                b12 = setupp.tile([128, NUC], F32)
                nc.vector.tensor_add(b12[:], b12a[:], b12b[:])

                # hidden^T as [128(d_in_chunk), dc, b]
                h_t = setupp.tile([128, NDC, BC], F32)
                for dc in range(NDC):
                    nc.sync.dma_start(
                        h_t[:, dc, :],
                        hidden.ap()[:, dc * 128 : (dc + 1) * 128].rearrange(
                            "b p -> p b"
                        ),
                    )

                w2c = setupp.tile([128, NDC, U], DTM)
                nc.vector.tensor_copy(w2c[:], w2_sb[:])
                h_tc = setupp.tile([128, NDC, BC], DTM)
                for dc in range(NDC):
                    nc.vector.tensor_copy(h_tc[:, dc, :], h_t[:, dc, :])

                # bias[u, b] = proj_h[b, u] + W1_b[u] + W2_b[u]  (fp32 matmul)
                for u in range(NUC):
                    ph = psmp.tile([128, BC], F32, tag="small")
                    for dc in range(NDC):
                        nc.tensor.matmul(
                            ph[:],
                            w2c[:, dc, u * 128 : (u + 1) * 128],
                            h_tc[:, dc, :],
                            start=(dc == 0),
                            stop=(dc == NDC - 1),
                        )
                    nc.vector.tensor_copy(phc[:, u, :], ph[:])
                    nc.vector.tensor_add(
                        bias_sb[:, u, :],
                        phc[:, u, :],
                        b12[:, u : u + 1].to_broadcast((128, BC)),
                    )

            # ACT absorber: make ScalarE observe the DVE tick of bias_sb so
            # every tanh carries only the PE wait (ISA structs hold one wait).
            bias_ab = nc.scalar.activation(
                ones_ad[:, 1:2], bias_sb[0:1, NUC - 1, 0:1],
                mybir.ActivationFunctionType.Identity,
            )

            # ---- main loop ----
            # Software-pipelined: half k+1's features load + rounding copy are
            # issued before half k's compute, so (a) the DVE tick of fr(k) is
            # already PE-observed when the transposes of half k issue (keeps
            # them at one wait), and (b) DMA overlaps compute.
            prev_attn_dma = None
            prev_zmm = [None]
            prev_sc = [None]
            frs = {}
            ft_last = {}       # half -> last ft copy inst
            ctx_dmas = {}      # example -> out_ctx dma inst
            _ab = [0]

            def dve_absorber(target_inst, reason):
                # A 1-element DVE copy that carries a wait the next copy's
                # ISA slot cannot hold. Rotating write slots keep its own
                # WAW ancient (and thus elided).
                i = _ab[0] % 16
                _ab[0] += 1
                inst = nc.vector.tensor_copy(
                    dve_dummy[0:1, i : i + 1], ones_ad[0:1, 0:1]
                )
                add_dep_helper(inst.ins, target_inst.ins, sync=True, reason=reason)
                return inst

            def load_half(k):
                bb, hh = divmod(k, 2)
                fn = fnatp.tile([128, 4, D], F32)
                fn_dma = nc.gpsimd.dma_start(
                    fn[:],
                    feats.ap()[bb, hh * TH : (hh + 1) * TH, :].rearrange(
                        "(c p) d -> p c d", p=128
                    ),
                )
                if prev_attn_dma is not None:
                    # SP-order the prefetch after the previous example's attn
                    # store: its DVE wait covers this slot's WAR, so this DMA
                    # carries only DMAHW waits (struct holds 2).
                    add_dep_helper(
                        fn_dma.ins, prev_attn_dma.ins, sync=False,
                        reason="order fn prefetch after prev attn out",
                    )
                fr = fnrp.tile([128, 4, D], DTM)
                nc.vector.tensor_copy(fr[:], fn[:])
                frs[k] = fr

            load_half(0)
            load_half(1)
            for b in range(BC):
                logit_row = rowsp.tile([1, T], F32, tag="logit")
                for h in range(2):
                    k = b * 2 + h
                    if k + 2 < BC * 2:
                        load_half(k + 2)
                    fr = frs[k]

                    def tr_group(dc):
                        # 4 transposes [t:128, d:128] -> ptr[:, 128] quarters
                        ptr = ptrp.tile([128, TH], DTM, tag=f"ptr{dc % 2}")
                        first = None
                        for c in range(4):
                            inst = nc.tensor.transpose(
                                ptr[:, c * 128 : (c + 1) * 128],
                                fr[:, c, dc * 128 : (dc + 1) * 128],
                                identc[:],
                            )
                            if first is None:
                                first = inst
                        ft = ftp.tile([128, TH], DTM, tag=f"ft{dc}")
                        ftc = nc.vector.tensor_copy(ft[:], ptr[:])
                        if ftab is not None:
                            add_dep_helper(
                                ftc.ins, ftab.ins, sync=False,
                                reason="DVE-order ft copy after absorber",
                            )
                        if dc == 3:
                            ft_last[k] = ftc
                        return ft, first

                    def mm_u0(dc, pp, fts):
                        return nc.tensor.matmul(
                            pp[:],
                            w1_r[:, dc, u0 * 128 : (u0 + 1) * 128],
                            fts[dc][:],
                            start=(dc == 0),
                            stop=False,
                        )

                    # Interleave: transposes of dc2/dc3 are PE-ordered after
                    # the first main matmuls, whose ft waits make their fr/ft
                    # WAR ticks already-observed (ISA structs hold one wait).
                    ftab = None
                    if k - 2 in ft_last:
                        ftab = dve_absorber(
                            ft_last[k - 2], "DVE observes ft copies of k-2"
                        )
                    fts = []
                    u0 = 0
                    for dc in (0, 1):
                        ft, tr_first = tr_group(dc)
                        fts.append(ft)
                        if prev_zmm[0] is not None:
                            add_dep_helper(
                                tr_first.ins, prev_zmm[0].ins, sync=False,
                                reason="PE-order transposes after prev zero-mm",
                            )
                    pp0 = ppp.tile([128, TH], F32, tag="pp")
                    mm00 = mm_u0(0, pp0, fts)
                    mm01 = mm_u0(1, pp0, fts)
                    for dc, mm in ((2, mm00), (3, mm01)):
                        ft, tr_first = tr_group(dc)
                        fts.append(ft)
                        add_dep_helper(
                            tr_first.ins, mm.ins, sync=False,
                            reason="PE-order transposes after u0 matmul",
                        )
                    # proj + tanh + V-reduce. The logit accumulation opens
                    # with a zero matmul (0.T @ fr of half k+1): it clears the
                    # PSUM bank, absorbs the slot WAR, and makes PE observe
                    # fr(k+1)'s DVE tick so the next half's transposes fit in
                    # their single wait slot.
                    pl = plp.tile([1, TH], F32)
                    vstart = True
                    if k + 1 < BC * 2:
                        zmm = nc.tensor.matmul(
                            pl[:],
                            zerocol[:],
                            frs[k + 1][:, 0, :],
                            start=True,
                            stop=False,
                        )
                        prev_zmm[0] = zmm
                        vstart = False
                    for u in range(NUC):
                        pp = pp0 if u == 0 else ppp.tile([128, TH], F32, tag="pp")
                        for dc in range(NDC):
                            if u == 0 and dc < 2:
                                continue
                            nc.tensor.matmul(
                                pp[:],
                                w1_r[:, dc, u * 128 : (u + 1) * 128],
                                fts[dc][:],
                                start=(u != 0 and dc == 0),
                                stop=(dc == NDC - 1),
                            )
                        sc = scorep.tile([128, TH], DTM)
                        if u % 2 == 0 and prev_sc[0] is not None:
                            # ACT absorber: carries the score-slot self-WAW
                            # (via an explicit sync dep on the previous tanh,
                            # reading no score tile) so each tanh keeps only
                            # its PE wait.
                            ab = nc.scalar.activation(
                                ones_ad[0:1, 1:2],
                                ones_ad[0:1, 0:1],
                                mybir.ActivationFunctionType.Identity,
                            )
                            add_dep_helper(
                                ab.ins, prev_sc[0].ins, sync=True,
                                reason="ACT observes prev tanh tick",
                            )
                        tanh_inst = nc.scalar.activation(
                            sc[:],
                            pp[:],
                            mybir.ActivationFunctionType.Tanh,
                            bias=bias_sb[:, u, b : b + 1],
                        )
                        if prev_sc[0] is None:
                            add_dep_helper(
                                tanh_inst.ins, bias_ab.ins, sync=False,
                                reason="ACT-order first tanh after bias absorber",
                            )
                        prev_sc[0] = tanh_inst
                        nc.tensor.matmul(
                            pl[:],
                            v_r[:, u, :],
                            sc[:],
                            start=(u == 0 and vstart),
                            stop=(u == NUC - 1),
                        )
                    nc.vector.tensor_copy(logit_row[:, h * TH : (h + 1) * TH], pl[:])

                # softmax over T (no max-subtract needed; |logit| <= sum|V|)
                exp_row = rowsp.tile([1, T], F32, tag=f"exp{b % 2}")
                sr = rowsp.tile([1, 2], F32, tag=f"sr{b % 2}")
                esum = sr[:, 0:1]
                rinv = sr[:, 1:2]
                nc.scalar.activation(
                    exp_row[:],
                    logit_row[:],
                    mybir.ActivationFunctionType.Exp,
                    accum_out=esum,
                )
                exp_c = rowsp.tile([1, T], F32, tag=f"expc{b % 2}")
                nc.vector.tensor_copy(exp_c[:], exp_row[:])
                nc.vector.reciprocal(rinv, esum)
                w_row = rowsp.tile([1, T], DTM, tag=f"wrow{b % 2}")
                nc.vector.tensor_mul(w_row[:], exp_c[:], rinv.to_broadcast((1, T)))
                w_row_b = rowsp.tile([1, T], BF16, tag=f"wrowb{b % 2}")
                nc.vector.tensor_copy(w_row_b[:], w_row[:])
                # interleaved 3D AP: the 3D pseudo-DMA struct holds two
                # waits (producer + lane); the contiguous 2D one holds one.
                prev_attn_dma = nc.gpsimd.dma_start(
                    out_attn.ap()[b : b + 1, :], w_row[:].bitcast(F32)
                )

                # transpose weights to [t:128, chunk:8] via K=1 matmuls.
                # A zero matmul (0.T @ zeros) opens the accumulation group:
                # it clears the bank and carries the slot WAW, so the first
                # data matmul waits only on w_row's DVE tick.
                pw = psmp.tile([128, 8], F32, tag="small")
                zpw = nc.tensor.matmul(
                    pw[:],
                    zrow_b[0:1, 0:128],
                    zrow_b[0:1, 0:8],
                    start=True,
                    stop=False,
                )
                first_wmm = None
                for c in range(8):
                    wmm = nc.tensor.matmul(
                        pw[:, c : c + 1],
                        w_row_b[0:1, c * 128 : (c + 1) * 128],
                        ones1_b[:],
                        start=False,
                        stop=(c == 7),
                    )
                    if first_wmm is None:
                        first_wmm = wmm
                        add_dep_helper(
                            wmm.ins, zpw.ins, sync=False,
                            reason="PE-order w-transpose after zero open",
                        )
                wt = rowsp.tile([128, 8], DTM, tag=f"wt{b % 2}")
                nc.vector.tensor_copy(wt[:], pw[:])

                # context: ctx[1, 512] += wT[t,1].T @ f_nat[t:128, d:512],
                # opened by a zero matmul (0.T @ logit_row slice) whose DVE
                # wait merges with the data matmuls' wt wait.
                pc = psmp.tile([1, D], F32, tag="small")
                zpc = nc.tensor.matmul(
                    pc[:],
                    zrow_b[0:1, 0:1],
                    zrow_b[0:1, 0:D],
                    start=True,
                    stop=False,
                )
                first_cmm = None
                for c in range(8):
                    cmm = nc.tensor.matmul(
                        pc[:],
                        wt[:, c : c + 1],
                        frs[b * 2 + c // 4][:, c % 4, :],
                        start=False,
                        stop=(c == 7),
                    )
                    if first_cmm is None:
                        first_cmm = cmm
                        add_dep_helper(
                            cmm.ins, zpc.ins, sync=False,
                            reason="PE-order ctx after zero open",
                        )
                ctx_row = rowsp.tile([1, D], F32, tag=f"ctxrow{b % 2}")
                if b - 2 in ctx_dmas:
                    cab = dve_absorber(
                        ctx_dmas[b - 2], "DVE observes out_ctx dma of b-2"
                    )
                    crc = nc.vector.tensor_copy(ctx_row[:], pc[:])
                    add_dep_helper(
                        crc.ins, cab.ins, sync=False,
                        reason="DVE-order ctx_row copy after absorber",
                    )
                else:
                    nc.vector.tensor_copy(ctx_row[:], pc[:])
                carrier = nc.gpsimd.dma_start(
                    ctx_scr.ap()[b : b + 1, :].rearrange("o (c p) -> o c p", p=64),
                    ctx_row.rearrange("o (c q) -> o c q", q=128)[:, :, 0:64],
                )
                ctx_dmas[b] = nc.gpsimd.dma_start(
                    out_ctx.ap()[b : b + 1, :], ctx_row[:]
                )
                add_dep_helper(
                    ctx_dmas[b].ins, carrier.ins, sync=False,
                    reason="Pool-order ctx store after carrier (ctx_row observed)",
                )
                frs.pop(b * 2, None)
                frs.pop(b * 2 + 1, None)

    return nc


def _split_excess_waits(nc, cap=1):
    """Walrus ISA structs hold very few semaphore waits per instruction
    (1 for most opcodes). Hoist all but `cap` waits of any instruction onto
    same-engine NoOps inserted immediately before it."""
    ctr = 0
    for fn in nc.m.functions:
        for blk in fn.blocks:
            insts = blk.instructions
            i = 0
            while i < len(insts):
                inst = insts[i]
                si = inst.sync_info
                if si is not None and si.on_wait and len(si.on_wait) > cap:
                    waits = list(si.on_wait)
                    for w in waits[:-cap]:
                        ctr += 1
                        nop = mybir.InstNoOp(
                            name=f"waitnop-{ctr}",
                            ins=[],
                            outs=[],
                            engine=inst.engine,
                            sync_info=mybir.SyncInfo(on_wait=[w], on_update=[]),
                        )
                        insts.insert(i, nop)
                        i += 1
                    inst.sync_info = mybir.SyncInfo(
                        on_wait=waits[-cap:], on_update=list(si.on_update)
                    )
                i += 1
    return ctr


def _get_graph(**kw):
    key = tuple(sorted(kw.items()))
    if key not in _CACHE:
        nc = build_graph(**kw)
        n = _split_excess_waits(nc)
        _CACHE[key] = nc
    return _CACHE[key]


def kernel(features, hidden, W1_w, W1_b, W2_w, W2_b, V_w, V_b=None, **ignored):
    nc = _get_graph()
    shared = {
        "W1_w": np.ascontiguousarray(W1_w, dtype=np.float32),
        "W1_b": np.ascontiguousarray(W1_b, dtype=np.float32),
        "W2_w": np.ascontiguousarray(W2_w, dtype=np.float32),
        "W2_b": np.ascontiguousarray(W2_b, dtype=np.float32),
        "V_w": np.ascontiguousarray(V_w, dtype=np.float32),
    }
    in_maps = []
    for c in range(NCORES):
        m = dict(shared)
        m["features"] = np.ascontiguousarray(
            features[c * BC : (c + 1) * BC], dtype=np.float32
        )
        m["hidden"] = np.ascontiguousarray(
            hidden[c * BC : (c + 1) * BC], dtype=np.float32
        )
        in_maps.append(m)
    res = run_bass_kernel_spmd(nc, in_maps, core_ids=list(range(NCORES)))
    ctx = np.concatenate([res.results[i]["out_ctx"] for i in range(NCORES)], axis=0)
    attn = np.concatenate(
        [res.results[i]["out_attn"] for i in range(NCORES)], axis=0
    ).reshape(B, T, 1)
    return ctx.astype(np.float32), attn.astype(np.float32)
